# revision 13
# baseline (speedup 1.0000x reference)
"""Two-layer GCN forward on 8 Trainium2 NeuronCores (Bass/Tile).

Strategy (graph/data parallel, dst-sharded):
  - Nodes sharded across 8 cores (12500/core, padded to 12544 = 98*128).
  - Per layer: sharded matmul h = x @ W, pre-scaled g = dinv * h, cast bf16,
    AllGather the per-node feature table to every core (256B rows).
  - Each core owns the edges whose dst lies in its shard. Per-edge work:
    dma_gather of g[src] rows (256B HBM reads) -> SBUF messages; a one-hot
    "selection" matrix built on the vector engine (dst_local == iota) turns
    the scatter-add into PE matmuls accumulating per-128-node dst block in
    PSUM.  out = relu(dinv*(acc + g_local) + b).
  - Self-loop term folded into the epilogue (g_local), never gathered.
  - Normalization dinv[src]*dinv[dst] is folded into per-node pre/post
    scaling, so the per-edge path is pure gather + matmul.

Edge indices are int16-limited (32768 rows/call), so the gathered table is
processed in 4 row-phases; host buckets each core's edges by
(phase, dst-block) with per-(phase,block) segment sizes padded to 128 and
shared across cores (SPMD: one program, per-core data).
"""

import numpy as np
import ml_dtypes

import concourse.bass as bass
import concourse.tile as tile
from concourse import bacc, mybir
from concourse.bass_utils import run_bass_kernel_spmd

NC = 8           # cores
P = 128          # partitions
ROWW = 128       # bf16 feature-table row width (256 bytes)
PHROWS = 32768   # gather index range per phase (int16 limit)
CHUNK = 1024     # max edge positions per gather call (SWDGE ring limit ~1-2K)
WIDE = 1024      # x-tile width for layer-1 matmul loads

BF16 = mybir.dt.bfloat16
F32 = mybir.dt.float32
I16 = mybir.dt.int16

DEBUG_SKIP_EDGE = False   # skip gather/matmul edge phases (crash bisect)
DEBUG_LOCAL_AG = False    # replace AllGather with local copy (crash bisect)
NSL = 8          # AllGather slices (one >~3MB collective wedges the device)


# ----------------------------------------------------------------- host prep

def _host_prep(x, edge_index, W1, b1, W2, b2):
    N, IN_DIM = x.shape
    HID = W1.shape[1]
    OUT = W2.shape[1]
    assert N % NC == 0
    SH = N // NC                      # real rows per shard
    SHP = -(-SH // P) * P             # padded rows per shard
    NBLK = SHP // P
    R = NC * SHP                      # padded table rows
    NPH = -(-R // PHROWS)

    src = np.asarray(edge_index[0], dtype=np.int64)
    dst = np.asarray(edge_index[1], dtype=np.int64)
    E = src.shape[0]

    deg = np.bincount(dst, minlength=N).astype(np.float64) + 1.0
    dinv = (1.0 / np.sqrt(deg)).astype(np.float32)

    # table row for each source node. The table is written by NSL sliced
    # AllGathers with contiguous outputs, so its layout is
    # [slice, core, rows-in-slice]: row = i*NC*SL + c*SL + r.
    SL = SHP // NSL
    assert SHP % NSL == 0
    sc = src // SH
    sa = src % SH
    srow = (sa // SL) * (NC * SL) + sc * SL + (sa % SL)
    phase = srow // PHROWS
    lidx = (srow % PHROWS).astype(np.int16)
    core = dst // SH
    blk = (dst % SH) // P
    dlo = ((dst % SH) % P).astype(np.int16)

    # group edges per (core, phase, block)
    order = np.lexsort((blk, phase, core))
    src_s, lidx_s, phase_s, core_s, blk_s, dlo_s = (
        src[order], lidx[order], phase[order], core[order], blk[order], dlo[order])

    # counts per (core, phase, block)
    key = (core_s * NPH + phase_s) * NBLK + blk_s
    cnt = np.bincount(key, minlength=NC * NPH * NBLK).reshape(NC, NPH, NBLK)
    Gsb = -(-cnt.max(axis=0) // P)            # [NPH, NBLK] groups, shared

    seg_pos = {}                              # (s, b) -> start position
    posn = 0
    segments = []                             # (s, b, ngroups) in emission order
    for s in range(NPH):
        for b in range(NBLK):
            g = int(Gsb[s, b])
            if g == 0:
                continue
            seg_pos[(s, b)] = posn
            segments.append((s, b, g))
            posn += g * P
    TTOT = posn
    assert TTOT % P == 0

    # chunks: pack segment pieces of one phase, <= CHUNK positions each.
    # A chunk's segs list holds (b, ngroups, first_piece, last_piece);
    # PSUM accumulation runs may span chunks (first/last flags drive
    # start= and the final accumulate into acc).
    chunks = []                               # (s, pos0, npos, [(b, ng, fst, lst)])
    cur = None
    for (s, b, g) in segments:
        gleft = g
        first = True
        while gleft > 0:
            if cur is not None and (cur[0] != s or cur[2] >= CHUNK):
                chunks.append(cur)
                cur = None
            if cur is None:
                cur = [s, seg_pos[(s, b)] + (g - gleft) * P, 0, []]
            take = min(gleft, (CHUNK - cur[2]) // P)
            cur[2] += take * P
            gleft -= take
            cur[3].append((b, take, first, gleft == 0))
            first = False
    if cur is not None:
        chunks.append(cur)

    # per-core position-indexed arrays
    starts = np.zeros(NC, np.int64)
    idx_all = np.zeros((NC, TTOT), np.int16)
    dlo_all = np.full((NC, TTOT), -1.0, np.float32)
    # edges of (c,s,b) occupy positions seg_pos[(s,b)] .. +cnt[c,s,b]
    csb_off = np.zeros(NC * NPH * NBLK + 1, np.int64)
    np.cumsum(cnt.reshape(-1), out=csb_off[1:])
    for c in range(NC):
        for s in range(NPH):
            for b in range(NBLK):
                n = int(cnt[c, s, b])
                if n == 0:
                    continue
                o = int(csb_off[(c * NPH + s) * NBLK + b])
                p0 = seg_pos[(s, b)]
                idx_all[c, p0:p0 + n] = lidx_s[o:o + n]
                dlo_all[c, p0:p0 + n] = dlo_s[o:o + n]
    del starts

    # wrap gather indices: [128, TTOT/16], 16-partition wrap replicated x8
    idx_w = np.ascontiguousarray(
        np.tile(idx_all.reshape(NC, TTOT // 16, 16).transpose(0, 2, 1), (1, 8, 1)))
    # dst values transposed: [128, TTOT/128]
    dst_t = np.ascontiguousarray(
        dlo_all.reshape(NC, TTOT // P, P).transpose(0, 2, 1)).astype(ml_dtypes.bfloat16)

    # x transposed & padded per core: [KCH, 128, SHP] bf16
    KCH = IN_DIM // P
    xtp = np.zeros((NC, KCH, P, SHP), ml_dtypes.bfloat16)
    xs = x.reshape(NC, SH, IN_DIM).astype(ml_dtypes.bfloat16)
    xtp[:, :, :, :SH] = xs.transpose(0, 2, 1).reshape(NC, KCH, P, SH)

    # W1 packed [128, KCH*HID] bf16
    w1p = np.ascontiguousarray(
        W1.reshape(KCH, P, HID).transpose(1, 0, 2).reshape(P, KCH * HID)
    ).astype(ml_dtypes.bfloat16)
    w2p = np.asarray(W2, np.float32)                       # [HID, OUT]

    dinv_pad = np.zeros((NC, SHP), np.float32)
    dinv_pad[:, :SH] = dinv.reshape(NC, SH)
    dinvc = np.ascontiguousarray(
        dinv_pad.reshape(NC, NBLK, P).transpose(0, 2, 1))  # [NC, 128, NBLK]

    b1r = np.tile(np.asarray(b1, np.float32)[None, :], (P, 1))
    b2r = np.tile(np.asarray(b2, np.float32)[None, :], (P, 1))
    iota = np.tile(np.arange(P, dtype=np.float32)[None, :], (P, CHUNK // P)
                   ).astype(ml_dtypes.bfloat16)
    ident = np.eye(P, dtype=np.float32)

    meta = dict(N=N, IN_DIM=IN_DIM, HID=HID, OUT=OUT, SH=SH, SHP=SHP,
                NBLK=NBLK, R=R, NPH=NPH, KCH=KCH, TTOT=TTOT, chunks=chunks)
    in_maps = []
    for c in range(NC):
        in_maps.append({
            "xt": np.ascontiguousarray(xtp[c]),
            "w1": w1p,
            "w2": w2p,
            "dinvc": np.ascontiguousarray(dinvc[c]),
            "b1r": b1r,
            "b2r": b2r,
            "iota": iota,
            "ident": ident,
            "gidx": idx_w[c],
            "dstv": dst_t[c],
        })
    return in_maps, meta


# ------------------------------------------------------------- device program

def _emit_edge_phase(nc, tc, stack_pools, meta, g_full, acc_ap, F,
                     dst_sb, iota_sb, gidx_dram, layer):
    """Gather + one-hot matmul accumulate for one layer. acc_ap: [128, NBLK*F]."""
    if DEBUG_SKIP_EDGE:
        return
    chunks = meta["chunks"]
    R = meta["R"]
    idxp, msgp, sp, psp = stack_pools
    open_ps = {}   # b -> (psum tile, n groups so far)
    for ci, (s, pos0, npos, segs) in enumerate(chunks):
        row0 = s * PHROWS
        row1 = min(row0 + PHROWS, R)
        idx_t = idxp.tile([P, npos // 16], I16, name=f"idx{layer}_{ci}", tag="idx")
        nc.sync.dma_start(idx_t[:], gidx_dram[:, pos0 // 16:(pos0 + npos) // 16])
        msgs = msgp.tile([P, npos // P, ROWW], BF16, name=f"msg{layer}_{ci}", tag="msgs")
        nc.gpsimd.dma_gather(
            out_ap=msgs[:],
            in_ap=g_full[row0:row1, :],
            idxs_ap=idx_t[:],
            num_idxs=npos,
            num_idxs_reg=npos,
            elem_size=ROWW,
        )
        S = sp.tile([P, npos], BF16, name=f"S{layer}_{ci}", tag="S")
        ngr = npos // P
        nc.vector.tensor_tensor(
            out=S[:].rearrange("p (g j) -> p g j", j=P),
            in0=dst_sb[:, pos0 // P:pos0 // P + ngr].to_broadcast([P, ngr, P]),
            in1=iota_sb[:, :npos].rearrange("p (g j) -> p g j", j=P),
            op=mybir.AluOpType.is_equal,
        )
        g = 0
        for (b, ng, fst, lst) in segs:
            if fst:
                ps = psp.tile([P, F], F32, space="PSUM",
                              name=f"ps{layer}_{ci}_{b}", tag="ps")
                done = 0
            else:
                ps, done = open_ps.pop(b)
            for i in range(ng):
                nc.tensor.matmul(
                    ps[:],
                    lhsT=S[:, (g + i) * P:(g + i + 1) * P],
                    rhs=msgs[:, g + i, :F],
                    start=(done + i == 0),
                    stop=(lst and i == ng - 1),
                )
            if lst:
                nc.vector.tensor_add(
                    acc_ap[:, b * F:(b + 1) * F],
                    acc_ap[:, b * F:(b + 1) * F], ps[:])
            else:
                open_ps[b] = (ps, done + ng)
            g += ng


def _build_program(meta):
    N, HID, OUT = meta["N"], meta["HID"], meta["OUT"]
    SHP, NBLK, R, KCH, TTOT = (meta["SHP"], meta["NBLK"], meta["R"],
                               meta["KCH"], meta["TTOT"])

    nc = bacc.Bacc("TRN2", target_bir_lowering=False, debug=False, num_devices=NC)

    t_xt = nc.dram_tensor("xt", [KCH, P, SHP], BF16, kind="ExternalInput")
    t_w1 = nc.dram_tensor("w1", [P, KCH * HID], BF16, kind="ExternalInput")
    t_w2 = nc.dram_tensor("w2", [HID, OUT], F32, kind="ExternalInput")
    t_dinvc = nc.dram_tensor("dinvc", [P, NBLK], F32, kind="ExternalInput")
    t_b1r = nc.dram_tensor("b1r", [P, HID], F32, kind="ExternalInput")
    t_b2r = nc.dram_tensor("b2r", [P, OUT], F32, kind="ExternalInput")
    t_iota = nc.dram_tensor("iota", [P, CHUNK], BF16, kind="ExternalInput")
    t_ident = nc.dram_tensor("ident", [P, P], F32, kind="ExternalInput")
    t_gidx = nc.dram_tensor("gidx", [P, TTOT // 16], I16, kind="ExternalInput")
    t_dstv = nc.dram_tensor("dstv", [P, TTOT // P], BF16, kind="ExternalInput")
    t_out = nc.dram_tensor("out", [SHP, OUT], F32, kind="ExternalOutput")

    g1_c = nc.dram_tensor("g1_c", [SHP, ROWW], BF16)
    g1_full = nc.dram_tensor("g1_full", [R, ROWW], BF16, addr_space="Shared")
    g2_c = nc.dram_tensor("g2_c", [SHP, ROWW], BF16)
    g2_full = nc.dram_tensor("g2_full", [R, ROWW], BF16, addr_space="Shared")
    o1t_d = nc.dram_tensor("o1t_d", [HID, SHP], F32)

    with tile.TileContext(nc) as tc:
        with tc.tile_pool(name="persist", bufs=1) as pers:
            w1_sb = pers.tile([P, KCH * HID], BF16)
            nc.sync.dma_start(w1_sb[:], t_w1[:])
            w2_sb = pers.tile([HID, OUT], F32)
            nc.sync.dma_start(w2_sb[:], t_w2[:])
            dinv_sb = pers.tile([P, NBLK], F32)
            nc.sync.dma_start(dinv_sb[:], t_dinvc[:])
            b1_sb = pers.tile([P, HID], F32)
            nc.sync.dma_start(b1_sb[:], t_b1r[:])
            b2_sb = pers.tile([P, OUT], F32)
            nc.sync.dma_start(b2_sb[:], t_b2r[:])
            iota_sb = pers.tile([P, CHUNK], BF16)
            nc.sync.dma_start(iota_sb[:], t_iota[:])
            ident_sb = pers.tile([P, P], F32)
            nc.sync.dma_start(ident_sb[:], t_ident[:])
            dst_sb = pers.tile([P, TTOT // P], BF16)
            nc.sync.dma_start(dst_sb[:], t_dstv[:])

            # ======== layer 1 scope (acc1/g1loc live M1 .. transpose) ========
            with tc.tile_pool(name="l1", bufs=1) as l1p:
                g1loc = l1p.tile([P, NBLK * HID], F32)
                acc1 = l1p.tile([P, NBLK * HID], F32)
                nc.vector.memset(acc1[:], 0.0)

                # ---- layer 1 matmul:  g1 = dinv * (x @ W1)
                with (tc.tile_pool(name="m1x", bufs=3) as xp,
                      tc.tile_pool(name="m1ps", bufs=4, space="PSUM") as m1psp,
                      tc.tile_pool(name="m1o", bufs=3) as m1op):
                    nwide = -(-SHP // WIDE)
                    for wi in range(nwide):
                        c0 = wi * WIDE
                        ncols = min(WIDE, SHP - c0)
                        xw = []
                        for k in range(KCH):
                            xt_k = xp.tile([P, ncols], BF16,
                                           name=f"xw{wi}_{k}", tag=f"xw{k}")
                            nc.sync.dma_start(xt_k[:], t_xt[k, :, c0:c0 + ncols])
                            xw.append(xt_k)
                        for rb in range(ncols // P):
                            gb = c0 // P + rb
                            ps = m1psp.tile([P, HID], F32, space="PSUM",
                                            name=f"m1ps{gb}", tag="m1ps")
                            for k in range(KCH):
                                nc.tensor.matmul(
                                    ps[:],
                                    lhsT=xw[k][:, rb * P:(rb + 1) * P],
                                    rhs=w1_sb[:, k * HID:(k + 1) * HID],
                                    start=(k == 0),
                                    stop=(k == KCH - 1),
                                )
                            nc.vector.tensor_scalar_mul(
                                g1loc[:, gb * HID:(gb + 1) * HID], ps[:],
                                dinv_sb[:, gb:gb + 1])
                            g1b = m1op.tile([P, HID], BF16,
                                            name=f"g1b{gb}", tag="g1b")
                            nc.vector.tensor_copy(
                                g1b[:], g1loc[:, gb * HID:(gb + 1) * HID])
                            nc.sync.dma_start(
                                g1_c[gb * P:(gb + 1) * P, 0:HID], g1b[:])

                # ---- AllGather layer-1 table (sliced; table layout
                # [slice, core, rows] so each collective output is contiguous)
                SL = SHP // NSL
                if DEBUG_LOCAL_AG:
                    nc.sync.dma_start(g1_full[0:SHP, :], g1_c[:])
                else:
                    for i in range(NSL):
                        nc.gpsimd.collective_compute(
                            "AllGather", mybir.AluOpType.bypass,
                            replica_groups=[list(range(NC))],
                            ins=[g1_c[i * SL:(i + 1) * SL, :]],
                            outs=[g1_full[i * NC * SL:(i + 1) * NC * SL, :]],
                        )

                # ---- layer 1 edge phase
                with (tc.tile_pool(name="e1idx", bufs=3) as idxp,
                      tc.tile_pool(name="e1msg", bufs=2) as msgp,
                      tc.tile_pool(name="e1S", bufs=2) as sp,
                      tc.tile_pool(name="e1ps", bufs=4, space="PSUM") as psp):
                    _emit_edge_phase(nc, tc, (idxp, msgp, sp, psp), meta,
                                     g1_full, acc1[:], HID, dst_sb, iota_sb,
                                     t_gidx, 1)

                # ---- layer-1 epilogue: out1 = relu(dinv*(acc+g1loc)+b1)
                a3 = acc1[:].rearrange("p (n h) -> p n h", h=HID)
                nc.vector.tensor_add(acc1[:], acc1[:], g1loc[:])
                nc.vector.tensor_tensor(
                    out=a3, in0=a3, in1=dinv_sb[:].to_broadcast([P, NBLK, HID]),
                    op=mybir.AluOpType.mult)
                nc.vector.tensor_tensor(
                    out=a3, in0=a3,
                    in1=b1_sb[:].to_broadcast([P, HID, NBLK]
                                              ).rearrange("p h n -> p n h"),
                    op=mybir.AluOpType.add)
                nc.vector.tensor_scalar_max(acc1[:], acc1[:], 0.0)

                # ---- transpose out1 -> o1t_d DRAM [HID, SHP]
                with (tc.tile_pool(name="tp", bufs=4, space="PSUM") as tpp,
                      tc.tile_pool(name="tpo", bufs=3) as tpo):
                    for gb in range(NBLK):
                        pst = tpp.tile([HID, P], F32, space="PSUM",
                                       name=f"pst{gb}", tag="pst")
                        nc.tensor.transpose(
                            pst[:], acc1[:, gb * HID:(gb + 1) * HID], ident_sb[:])
                        o1s = tpo.tile([HID, P], F32, name=f"o1s{gb}", tag="o1s")
                        nc.vector.tensor_copy(o1s[:], pst[:])
                        nc.sync.dma_start(o1t_d[:, gb * P:(gb + 1) * P], o1s[:])

            # ======== layer 2 scope ========
            with tc.tile_pool(name="l2", bufs=1) as l2p:
                g2loc = l2p.tile([P, NBLK * OUT], F32)
                acc2 = l2p.tile([P, NBLK * OUT], F32)
                nc.vector.memset(acc2[:], 0.0)

                # ---- layer-2 matmul: g2 = dinv * (out1 @ W2)
                with (tc.tile_pool(name="m2x", bufs=3) as o1xp,
                      tc.tile_pool(name="m2ps", bufs=4, space="PSUM") as m2psp,
                      tc.tile_pool(name="m2o", bufs=3) as m2op):
                    nwide = -(-SHP // WIDE)
                    for wi in range(nwide):
                        c0 = wi * WIDE
                        ncols = min(WIDE, SHP - c0)
                        o1w = o1xp.tile([HID, ncols], F32,
                                        name=f"o1w{wi}", tag="o1w")
                        nc.sync.dma_start(o1w[:], o1t_d[:, c0:c0 + ncols])
                        for rb in range(ncols // P):
                            gb = c0 // P + rb
                            ps2 = m2psp.tile([P, OUT], F32, space="PSUM",
                                             name=f"m2ps{gb}", tag="m2ps")
                            nc.tensor.matmul(
                                ps2[:], lhsT=o1w[:, rb * P:(rb + 1) * P],
                                rhs=w2_sb[:], start=True, stop=True)
                            nc.vector.tensor_scalar_mul(
                                g2loc[:, gb * OUT:(gb + 1) * OUT], ps2[:],
                                dinv_sb[:, gb:gb + 1])
                            g2b = m2op.tile([P, OUT], BF16,
                                            name=f"g2b{gb}", tag="g2b")
                            nc.vector.tensor_copy(
                                g2b[:], g2loc[:, gb * OUT:(gb + 1) * OUT])
                            nc.sync.dma_start(
                                g2_c[gb * P:(gb + 1) * P, 0:OUT], g2b[:])

                SL = SHP // NSL
                if DEBUG_LOCAL_AG:
                    nc.sync.dma_start(g2_full[0:SHP, :], g2_c[:])
                else:
                    for i in range(NSL):
                        nc.gpsimd.collective_compute(
                            "AllGather", mybir.AluOpType.bypass,
                            replica_groups=[list(range(NC))],
                            ins=[g2_c[i * SL:(i + 1) * SL, :]],
                            outs=[g2_full[i * NC * SL:(i + 1) * NC * SL, :]],
                        )

                # ---- layer 2 edge phase
                with (tc.tile_pool(name="e2idx", bufs=3) as idxp,
                      tc.tile_pool(name="e2msg", bufs=2) as msgp,
                      tc.tile_pool(name="e2S", bufs=2) as sp,
                      tc.tile_pool(name="e2ps", bufs=4, space="PSUM") as psp):
                    _emit_edge_phase(nc, tc, (idxp, msgp, sp, psp), meta,
                                     g2_full, acc2[:], OUT, dst_sb, iota_sb,
                                     t_gidx, 2)

                # ---- layer-2 epilogue: out = dinv*(acc2+g2loc)+b2
                c3 = acc2[:].rearrange("p (n h) -> p n h", h=OUT)
                nc.vector.tensor_add(acc2[:], acc2[:], g2loc[:])
                nc.vector.tensor_tensor(
                    out=c3, in0=c3, in1=dinv_sb[:].to_broadcast([P, NBLK, OUT]),
                    op=mybir.AluOpType.mult)
                nc.vector.tensor_tensor(
                    out=c3, in0=c3,
                    in1=b2_sb[:].to_broadcast([P, OUT, NBLK]
                                              ).rearrange("p h n -> p n h"),
                    op=mybir.AluOpType.add)
                for gb in range(NBLK):
                    nc.sync.dma_start(
                        t_out[gb * P:(gb + 1) * P, :],
                        acc2[:, gb * OUT:(gb + 1) * OUT])

    nc.compile()
    return nc


# ------------------------------------------------------------------ frontend

_CACHE = {}


def run(trace=False, **inputs):
    in_maps, meta = _host_prep(
        inputs["x"], inputs["edge_index"], inputs["W1"], inputs["b1"],
        inputs["W2"], inputs["b2"])
    key = (meta["N"], meta["IN_DIM"], meta["HID"], meta["OUT"], meta["TTOT"],
           tuple((s, p, n, tuple(sg)) for s, p, n, sg in meta["chunks"]))
    if key not in _CACHE:
        _CACHE.clear()
        _CACHE[key] = _build_program(meta)
    nc = _CACHE[key]
    res = run_bass_kernel_spmd(nc, in_maps, list(range(NC)), trace=trace)
    SH = meta["SH"]
    out = np.concatenate([res.results[c]["out"][:SH] for c in range(NC)], axis=0)
    return out.astype(np.float32), res


def kernel(**inputs):
    out, _ = run(trace=False, **inputs)
    return out


# revision 15
# speedup vs baseline: 1.0531x; 1.0531x over previous
"""Two-layer GCN forward on 8 Trainium2 NeuronCores (Bass/Tile).

Strategy (graph/data parallel, dst-sharded):
  - Nodes sharded across 8 cores (12500/core, padded to 12544 = 98*128).
  - Per layer: sharded matmul h = x @ W, pre-scaled g = dinv * h, cast bf16,
    AllGather the per-node feature table to every core (256B rows).
  - Each core owns the edges whose dst lies in its shard. Per-edge work:
    dma_gather of g[src] rows (256B HBM reads) -> SBUF messages; a one-hot
    "selection" matrix built on the vector engine (dst_local == iota) turns
    the scatter-add into PE matmuls accumulating per-128-node dst block in
    PSUM.  out = relu(dinv*(acc + g_local) + b).
  - Self-loop term folded into the epilogue (g_local), never gathered.
  - Normalization dinv[src]*dinv[dst] is folded into per-node pre/post
    scaling, so the per-edge path is pure gather + matmul.

Edge indices are int16-limited (32768 rows/call), so the gathered table is
processed in 4 row-phases; host buckets each core's edges by
(phase, dst-block) with per-(phase,block) segment sizes padded to 128 and
shared across cores (SPMD: one program, per-core data).
"""

import numpy as np
import ml_dtypes

import concourse.bass as bass
import concourse.tile as tile
from concourse import bacc, mybir
from concourse.bass_utils import run_bass_kernel_spmd

NC = 8           # cores
P = 128          # partitions
ROWW = 128       # bf16 feature-table row width (256 bytes)
PHROWS = 32768   # gather index range per phase (int16 limit)
CHUNK = 4096     # edge positions per gather call (needs 64K SWDGE scratch + multi-packet)
WIDE = 1024      # x-tile width for layer-1 matmul loads

BF16 = mybir.dt.bfloat16
F32 = mybir.dt.float32
I16 = mybir.dt.int16

DEBUG_SKIP_EDGE = False   # skip gather/matmul edge phases (crash bisect)
DEBUG_LOCAL_AG = False    # replace AllGather with local copy (crash bisect)
NSL = 8          # AllGather slices (one >~3MB collective wedges the device)
DMA_SCRATCH = 65536   # SWDGE descriptor-ring carveout (bytes)
SINGLE_PACKET = False  # multi-packet: one call's descs may exceed one ring packet


# ----------------------------------------------------------------- host prep

def _host_prep(x, edge_index, W1, b1, W2, b2):
    N, IN_DIM = x.shape
    HID = W1.shape[1]
    OUT = W2.shape[1]
    assert N % NC == 0
    SH = N // NC                      # real rows per shard
    SHP = -(-SH // P) * P             # padded rows per shard
    NBLK = SHP // P
    R = NC * SHP                      # padded table rows
    NPH = -(-R // PHROWS)

    src = np.asarray(edge_index[0], dtype=np.int64)
    dst = np.asarray(edge_index[1], dtype=np.int64)
    E = src.shape[0]

    deg = np.bincount(dst, minlength=N).astype(np.float64) + 1.0
    dinv = (1.0 / np.sqrt(deg)).astype(np.float32)

    # table row for each source node. The table is written by NSL sliced
    # AllGathers with contiguous outputs, so its layout is
    # [slice, core, rows-in-slice]: row = i*NC*SL + c*SL + r.
    SL = SHP // NSL
    assert SHP % NSL == 0
    sc = src // SH
    sa = src % SH
    srow = (sa // SL) * (NC * SL) + sc * SL + (sa % SL)
    phase = srow // PHROWS
    lidx = (srow % PHROWS).astype(np.int16)
    core = dst // SH
    blk = (dst % SH) // P
    dlo = ((dst % SH) % P).astype(np.int16)

    # group edges per (core, phase, block)
    order = np.lexsort((blk, phase, core))
    src_s, lidx_s, phase_s, core_s, blk_s, dlo_s = (
        src[order], lidx[order], phase[order], core[order], blk[order], dlo[order])

    # counts per (core, phase, block)
    key = (core_s * NPH + phase_s) * NBLK + blk_s
    cnt = np.bincount(key, minlength=NC * NPH * NBLK).reshape(NC, NPH, NBLK)
    Gsb = -(-cnt.max(axis=0) // P)            # [NPH, NBLK] groups, shared

    seg_pos = {}                              # (s, b) -> start position
    posn = 0
    segments = []                             # (s, b, ngroups) in emission order
    for s in range(NPH):
        for b in range(NBLK):
            g = int(Gsb[s, b])
            if g == 0:
                continue
            seg_pos[(s, b)] = posn
            segments.append((s, b, g))
            posn += g * P
    TTOT = posn
    assert TTOT % P == 0

    # chunks: pack segment pieces of one phase, <= CHUNK positions each.
    # A chunk's segs list holds (b, ngroups, first_piece, last_piece);
    # PSUM accumulation runs may span chunks (first/last flags drive
    # start= and the final accumulate into acc).
    chunks = []                               # (s, pos0, npos, [(b, ng, fst, lst)])
    cur = None
    for (s, b, g) in segments:
        gleft = g
        first = True
        while gleft > 0:
            if cur is not None and (cur[0] != s or cur[2] >= CHUNK):
                chunks.append(cur)
                cur = None
            if cur is None:
                cur = [s, seg_pos[(s, b)] + (g - gleft) * P, 0, []]
            take = min(gleft, (CHUNK - cur[2]) // P)
            cur[2] += take * P
            gleft -= take
            cur[3].append((b, take, first, gleft == 0))
            first = False
    if cur is not None:
        chunks.append(cur)

    # per-core position-indexed arrays
    starts = np.zeros(NC, np.int64)
    idx_all = np.zeros((NC, TTOT), np.int16)
    dlo_all = np.full((NC, TTOT), -1.0, np.float32)
    # edges of (c,s,b) occupy positions seg_pos[(s,b)] .. +cnt[c,s,b]
    csb_off = np.zeros(NC * NPH * NBLK + 1, np.int64)
    np.cumsum(cnt.reshape(-1), out=csb_off[1:])
    for c in range(NC):
        for s in range(NPH):
            for b in range(NBLK):
                n = int(cnt[c, s, b])
                if n == 0:
                    continue
                o = int(csb_off[(c * NPH + s) * NBLK + b])
                p0 = seg_pos[(s, b)]
                idx_all[c, p0:p0 + n] = lidx_s[o:o + n]
                dlo_all[c, p0:p0 + n] = dlo_s[o:o + n]
    del starts

    # wrap gather indices: [128, TTOT/16], 16-partition wrap replicated x8
    idx_w = np.ascontiguousarray(
        np.tile(idx_all.reshape(NC, TTOT // 16, 16).transpose(0, 2, 1), (1, 8, 1)))
    # dst values transposed: [128, TTOT/128]
    dst_t = np.ascontiguousarray(
        dlo_all.reshape(NC, TTOT // P, P).transpose(0, 2, 1)).astype(ml_dtypes.bfloat16)

    # x transposed & padded per core: [KCH, 128, SHP] bf16
    KCH = IN_DIM // P
    xtp = np.zeros((NC, KCH, P, SHP), ml_dtypes.bfloat16)
    xs = x.reshape(NC, SH, IN_DIM).astype(ml_dtypes.bfloat16)
    xtp[:, :, :, :SH] = xs.transpose(0, 2, 1).reshape(NC, KCH, P, SH)

    # W1 packed [128, KCH*HID] bf16
    w1p = np.ascontiguousarray(
        W1.reshape(KCH, P, HID).transpose(1, 0, 2).reshape(P, KCH * HID)
    ).astype(ml_dtypes.bfloat16)
    w2p = np.asarray(W2, np.float32)                       # [HID, OUT]

    dinv_pad = np.zeros((NC, SHP), np.float32)
    dinv_pad[:, :SH] = dinv.reshape(NC, SH)
    dinvc = np.ascontiguousarray(
        dinv_pad.reshape(NC, NBLK, P).transpose(0, 2, 1))  # [NC, 128, NBLK]

    b1r = np.tile(np.asarray(b1, np.float32)[None, :], (P, 1))
    b2r = np.tile(np.asarray(b2, np.float32)[None, :], (P, 1))
    iota = np.tile(np.arange(P, dtype=np.float32)[None, :], (P, CHUNK // P)
                   ).astype(ml_dtypes.bfloat16)
    ident = np.eye(P, dtype=np.float32)

    meta = dict(N=N, IN_DIM=IN_DIM, HID=HID, OUT=OUT, SH=SH, SHP=SHP,
                NBLK=NBLK, R=R, NPH=NPH, KCH=KCH, TTOT=TTOT, chunks=chunks)
    in_maps = []
    for c in range(NC):
        in_maps.append({
            "xt": np.ascontiguousarray(xtp[c]),
            "w1": w1p,
            "w2": w2p,
            "dinvc": np.ascontiguousarray(dinvc[c]),
            "b1r": b1r,
            "b2r": b2r,
            "iota": iota,
            "ident": ident,
            "gidx": idx_w[c],
            "dstv": dst_t[c],
        })
    return in_maps, meta


# ------------------------------------------------------------- device program

def _emit_edge_phase(nc, tc, stack_pools, meta, g_full, acc_ap, F,
                     dst_sb, iota_sb, gidx_dram, layer):
    """Gather + one-hot matmul accumulate for one layer. acc_ap: [128, NBLK*F]."""
    if DEBUG_SKIP_EDGE:
        return
    chunks = meta["chunks"]
    R = meta["R"]
    idxp, msgp, sp, psp = stack_pools
    open_ps = {}   # b -> (psum tile, n groups so far)
    for ci, (s, pos0, npos, segs) in enumerate(chunks):
        row0 = s * PHROWS
        row1 = min(row0 + PHROWS, R)
        idx_t = idxp.tile([P, npos // 16], I16, name=f"idx{layer}_{ci}", tag="idx")
        nc.sync.dma_start(idx_t[:], gidx_dram[:, pos0 // 16:(pos0 + npos) // 16])
        msgs = msgp.tile([P, npos // P, ROWW], BF16, name=f"msg{layer}_{ci}", tag="msgs")
        nc.gpsimd.dma_gather(
            out_ap=msgs[:],
            in_ap=g_full[row0:row1, :],
            idxs_ap=idx_t[:],
            num_idxs=npos,
            num_idxs_reg=npos,
            elem_size=ROWW,
            single_packet=SINGLE_PACKET,
        )
        S = sp.tile([P, npos], BF16, name=f"S{layer}_{ci}", tag="S")
        ngr = npos // P
        nc.vector.tensor_tensor(
            out=S[:].rearrange("p (g j) -> p g j", j=P),
            in0=dst_sb[:, pos0 // P:pos0 // P + ngr].to_broadcast([P, ngr, P]),
            in1=iota_sb[:, :npos].rearrange("p (g j) -> p g j", j=P),
            op=mybir.AluOpType.is_equal,
        )
        g = 0
        for (b, ng, fst, lst) in segs:
            if fst:
                ps = psp.tile([P, F], F32, space="PSUM",
                              name=f"ps{layer}_{ci}_{b}", tag="ps")
                done = 0
            else:
                ps, done = open_ps.pop(b)
            for i in range(ng):
                nc.tensor.matmul(
                    ps[:],
                    lhsT=S[:, (g + i) * P:(g + i + 1) * P],
                    rhs=msgs[:, g + i, :F],
                    start=(done + i == 0),
                    stop=(lst and i == ng - 1),
                )
            if lst:
                nc.vector.tensor_add(
                    acc_ap[:, b * F:(b + 1) * F],
                    acc_ap[:, b * F:(b + 1) * F], ps[:])
            else:
                open_ps[b] = (ps, done + ng)
            g += ng


def _build_program(meta):
    N, HID, OUT = meta["N"], meta["HID"], meta["OUT"]
    SHP, NBLK, R, KCH, TTOT = (meta["SHP"], meta["NBLK"], meta["R"],
                               meta["KCH"], meta["TTOT"])

    nc = bacc.Bacc("TRN2", target_bir_lowering=False, debug=False, num_devices=NC,
                   dynamic_dma_scratch_size=DMA_SCRATCH)

    t_xt = nc.dram_tensor("xt", [KCH, P, SHP], BF16, kind="ExternalInput")
    t_w1 = nc.dram_tensor("w1", [P, KCH * HID], BF16, kind="ExternalInput")
    t_w2 = nc.dram_tensor("w2", [HID, OUT], F32, kind="ExternalInput")
    t_dinvc = nc.dram_tensor("dinvc", [P, NBLK], F32, kind="ExternalInput")
    t_b1r = nc.dram_tensor("b1r", [P, HID], F32, kind="ExternalInput")
    t_b2r = nc.dram_tensor("b2r", [P, OUT], F32, kind="ExternalInput")
    t_iota = nc.dram_tensor("iota", [P, CHUNK], BF16, kind="ExternalInput")
    t_ident = nc.dram_tensor("ident", [P, P], F32, kind="ExternalInput")
    t_gidx = nc.dram_tensor("gidx", [P, TTOT // 16], I16, kind="ExternalInput")
    t_dstv = nc.dram_tensor("dstv", [P, TTOT // P], BF16, kind="ExternalInput")
    t_out = nc.dram_tensor("out", [SHP, OUT], F32, kind="ExternalOutput")

    g1_c = nc.dram_tensor("g1_c", [SHP, ROWW], BF16)
    g1_full = nc.dram_tensor("g1_full", [R, ROWW], BF16, addr_space="Shared")
    g2_c = nc.dram_tensor("g2_c", [SHP, ROWW], BF16)
    g2_full = nc.dram_tensor("g2_full", [R, ROWW], BF16, addr_space="Shared")
    o1t_d = nc.dram_tensor("o1t_d", [HID, SHP], F32)

    with tile.TileContext(nc) as tc:
        with tc.tile_pool(name="persist", bufs=1) as pers:
            w1_sb = pers.tile([P, KCH * HID], BF16)
            nc.sync.dma_start(w1_sb[:], t_w1[:])
            w2_sb = pers.tile([HID, OUT], F32)
            nc.sync.dma_start(w2_sb[:], t_w2[:])
            dinv_sb = pers.tile([P, NBLK], F32)
            nc.sync.dma_start(dinv_sb[:], t_dinvc[:])
            b1_sb = pers.tile([P, HID], F32)
            nc.sync.dma_start(b1_sb[:], t_b1r[:])
            b2_sb = pers.tile([P, OUT], F32)
            nc.sync.dma_start(b2_sb[:], t_b2r[:])
            iota_sb = pers.tile([P, CHUNK], BF16)
            nc.sync.dma_start(iota_sb[:], t_iota[:])
            ident_sb = pers.tile([P, P], F32)
            nc.sync.dma_start(ident_sb[:], t_ident[:])
            dst_sb = pers.tile([P, TTOT // P], BF16)
            nc.sync.dma_start(dst_sb[:], t_dstv[:])

            # ======== layer 1 scope (acc1/g1loc live M1 .. transpose) ========
            with tc.tile_pool(name="l1", bufs=1) as l1p:
                g1loc = l1p.tile([P, NBLK * HID], F32)
                acc1 = l1p.tile([P, NBLK * HID], F32)
                nc.vector.memset(acc1[:], 0.0)

                # ---- layer 1 matmul:  g1 = dinv * (x @ W1)
                with (tc.tile_pool(name="m1x", bufs=3) as xp,
                      tc.tile_pool(name="m1ps", bufs=4, space="PSUM") as m1psp,
                      tc.tile_pool(name="m1o", bufs=3) as m1op):
                    nwide = -(-SHP // WIDE)
                    for wi in range(nwide):
                        c0 = wi * WIDE
                        ncols = min(WIDE, SHP - c0)
                        xw = []
                        for k in range(KCH):
                            xt_k = xp.tile([P, ncols], BF16,
                                           name=f"xw{wi}_{k}", tag=f"xw{k}")
                            nc.sync.dma_start(xt_k[:], t_xt[k, :, c0:c0 + ncols])
                            xw.append(xt_k)
                        for rb in range(ncols // P):
                            gb = c0 // P + rb
                            ps = m1psp.tile([P, HID], F32, space="PSUM",
                                            name=f"m1ps{gb}", tag="m1ps")
                            for k in range(KCH):
                                nc.tensor.matmul(
                                    ps[:],
                                    lhsT=xw[k][:, rb * P:(rb + 1) * P],
                                    rhs=w1_sb[:, k * HID:(k + 1) * HID],
                                    start=(k == 0),
                                    stop=(k == KCH - 1),
                                )
                            nc.vector.tensor_scalar_mul(
                                g1loc[:, gb * HID:(gb + 1) * HID], ps[:],
                                dinv_sb[:, gb:gb + 1])
                            g1b = m1op.tile([P, HID], BF16,
                                            name=f"g1b{gb}", tag="g1b")
                            nc.vector.tensor_copy(
                                g1b[:], g1loc[:, gb * HID:(gb + 1) * HID])
                            nc.sync.dma_start(
                                g1_c[gb * P:(gb + 1) * P, 0:HID], g1b[:])

                # ---- AllGather layer-1 table (sliced; table layout
                # [slice, core, rows] so each collective output is contiguous)
                SL = SHP // NSL
                if DEBUG_LOCAL_AG:
                    nc.sync.dma_start(g1_full[0:SHP, :], g1_c[:])
                else:
                    for i in range(NSL):
                        nc.gpsimd.collective_compute(
                            "AllGather", mybir.AluOpType.bypass,
                            replica_groups=[list(range(NC))],
                            ins=[g1_c[i * SL:(i + 1) * SL, :]],
                            outs=[g1_full[i * NC * SL:(i + 1) * NC * SL, :]],
                        )

                # ---- layer 1 edge phase
                with (tc.tile_pool(name="e1idx", bufs=3) as idxp,
                      tc.tile_pool(name="e1msg", bufs=2) as msgp,
                      tc.tile_pool(name="e1S", bufs=2) as sp,
                      tc.tile_pool(name="e1ps", bufs=4, space="PSUM") as psp):
                    _emit_edge_phase(nc, tc, (idxp, msgp, sp, psp), meta,
                                     g1_full, acc1[:], HID, dst_sb, iota_sb,
                                     t_gidx, 1)

                # ---- layer-1 epilogue: out1 = relu(dinv*(acc+g1loc)+b1)
                a3 = acc1[:].rearrange("p (n h) -> p n h", h=HID)
                nc.vector.tensor_add(acc1[:], acc1[:], g1loc[:])
                nc.vector.tensor_tensor(
                    out=a3, in0=a3, in1=dinv_sb[:].to_broadcast([P, NBLK, HID]),
                    op=mybir.AluOpType.mult)
                nc.vector.tensor_tensor(
                    out=a3, in0=a3,
                    in1=b1_sb[:].to_broadcast([P, HID, NBLK]
                                              ).rearrange("p h n -> p n h"),
                    op=mybir.AluOpType.add)
                nc.vector.tensor_scalar_max(acc1[:], acc1[:], 0.0)

                # ---- transpose out1 -> o1t_d DRAM [HID, SHP]
                with (tc.tile_pool(name="tp", bufs=4, space="PSUM") as tpp,
                      tc.tile_pool(name="tpo", bufs=3) as tpo):
                    for gb in range(NBLK):
                        pst = tpp.tile([HID, P], F32, space="PSUM",
                                       name=f"pst{gb}", tag="pst")
                        nc.tensor.transpose(
                            pst[:], acc1[:, gb * HID:(gb + 1) * HID], ident_sb[:])
                        o1s = tpo.tile([HID, P], F32, name=f"o1s{gb}", tag="o1s")
                        nc.vector.tensor_copy(o1s[:], pst[:])
                        nc.sync.dma_start(o1t_d[:, gb * P:(gb + 1) * P], o1s[:])

            # ======== layer 2 scope ========
            with tc.tile_pool(name="l2", bufs=1) as l2p:
                g2loc = l2p.tile([P, NBLK * OUT], F32)
                acc2 = l2p.tile([P, NBLK * OUT], F32)
                nc.vector.memset(acc2[:], 0.0)

                # ---- layer-2 matmul: g2 = dinv * (out1 @ W2)
                with (tc.tile_pool(name="m2x", bufs=3) as o1xp,
                      tc.tile_pool(name="m2ps", bufs=4, space="PSUM") as m2psp,
                      tc.tile_pool(name="m2o", bufs=3) as m2op):
                    nwide = -(-SHP // WIDE)
                    for wi in range(nwide):
                        c0 = wi * WIDE
                        ncols = min(WIDE, SHP - c0)
                        o1w = o1xp.tile([HID, ncols], F32,
                                        name=f"o1w{wi}", tag="o1w")
                        nc.sync.dma_start(o1w[:], o1t_d[:, c0:c0 + ncols])
                        for rb in range(ncols // P):
                            gb = c0 // P + rb
                            ps2 = m2psp.tile([P, OUT], F32, space="PSUM",
                                             name=f"m2ps{gb}", tag="m2ps")
                            nc.tensor.matmul(
                                ps2[:], lhsT=o1w[:, rb * P:(rb + 1) * P],
                                rhs=w2_sb[:], start=True, stop=True)
                            nc.vector.tensor_scalar_mul(
                                g2loc[:, gb * OUT:(gb + 1) * OUT], ps2[:],
                                dinv_sb[:, gb:gb + 1])
                            g2b = m2op.tile([P, OUT], BF16,
                                            name=f"g2b{gb}", tag="g2b")
                            nc.vector.tensor_copy(
                                g2b[:], g2loc[:, gb * OUT:(gb + 1) * OUT])
                            nc.sync.dma_start(
                                g2_c[gb * P:(gb + 1) * P, 0:OUT], g2b[:])

                SL = SHP // NSL
                if DEBUG_LOCAL_AG:
                    nc.sync.dma_start(g2_full[0:SHP, :], g2_c[:])
                else:
                    for i in range(NSL):
                        nc.gpsimd.collective_compute(
                            "AllGather", mybir.AluOpType.bypass,
                            replica_groups=[list(range(NC))],
                            ins=[g2_c[i * SL:(i + 1) * SL, :]],
                            outs=[g2_full[i * NC * SL:(i + 1) * NC * SL, :]],
                        )

                # ---- layer 2 edge phase
                with (tc.tile_pool(name="e2idx", bufs=3) as idxp,
                      tc.tile_pool(name="e2msg", bufs=2) as msgp,
                      tc.tile_pool(name="e2S", bufs=2) as sp,
                      tc.tile_pool(name="e2ps", bufs=4, space="PSUM") as psp):
                    _emit_edge_phase(nc, tc, (idxp, msgp, sp, psp), meta,
                                     g2_full, acc2[:], OUT, dst_sb, iota_sb,
                                     t_gidx, 2)

                # ---- layer-2 epilogue: out = dinv*(acc2+g2loc)+b2
                c3 = acc2[:].rearrange("p (n h) -> p n h", h=OUT)
                nc.vector.tensor_add(acc2[:], acc2[:], g2loc[:])
                nc.vector.tensor_tensor(
                    out=c3, in0=c3, in1=dinv_sb[:].to_broadcast([P, NBLK, OUT]),
                    op=mybir.AluOpType.mult)
                nc.vector.tensor_tensor(
                    out=c3, in0=c3,
                    in1=b2_sb[:].to_broadcast([P, OUT, NBLK]
                                              ).rearrange("p h n -> p n h"),
                    op=mybir.AluOpType.add)
                for gb in range(NBLK):
                    nc.sync.dma_start(
                        t_out[gb * P:(gb + 1) * P, :],
                        acc2[:, gb * OUT:(gb + 1) * OUT])

    nc.compile()
    return nc


# ------------------------------------------------------------------ frontend

_CACHE = {}


def run(trace=False, **inputs):
    in_maps, meta = _host_prep(
        inputs["x"], inputs["edge_index"], inputs["W1"], inputs["b1"],
        inputs["W2"], inputs["b2"])
    key = (meta["N"], meta["IN_DIM"], meta["HID"], meta["OUT"], meta["TTOT"],
           tuple((s, p, n, tuple(sg)) for s, p, n, sg in meta["chunks"]))
    if key not in _CACHE:
        _CACHE.clear()
        _CACHE[key] = _build_program(meta)
    nc = _CACHE[key]
    res = run_bass_kernel_spmd(nc, in_maps, list(range(NC)), trace=trace)
    SH = meta["SH"]
    out = np.concatenate([res.results[c]["out"][:SH] for c in range(NC)], axis=0)
    return out.astype(np.float32), res


def kernel(**inputs):
    out, _ = run(trace=False, **inputs)
    return out


# revision 16
# speedup vs baseline: 1.3689x; 1.2999x over previous
"""Two-layer GCN forward on 8 Trainium2 NeuronCores (Bass/Tile).

Strategy (graph/data parallel, dst-sharded):
  - Nodes sharded across 8 cores (12500/core, padded to 12544 = 98*128).
  - Per layer: sharded matmul h = x @ W, pre-scaled g = dinv * h, cast bf16,
    AllGather the per-node feature table to every core (256B rows).
  - Each core owns the edges whose dst lies in its shard. Per-edge work:
    dma_gather of g[src] rows (256B HBM reads) -> SBUF messages; a one-hot
    "selection" matrix built on the vector engine (dst_local == iota) turns
    the scatter-add into PE matmuls accumulating per-128-node dst block in
    PSUM.  out = relu(dinv*(acc + g_local) + b).
  - Self-loop term folded into the epilogue (g_local), never gathered.
  - Normalization dinv[src]*dinv[dst] is folded into per-node pre/post
    scaling, so the per-edge path is pure gather + matmul.

Edge indices are int16-limited (32768 rows/call), so the gathered table is
processed in 4 row-phases; host buckets each core's edges by
(phase, dst-block) with per-(phase,block) segment sizes padded to 128 and
shared across cores (SPMD: one program, per-core data).
"""

import numpy as np
import ml_dtypes

import concourse.bass as bass
import concourse.tile as tile
from concourse import bacc, mybir
from concourse.bass_utils import run_bass_kernel_spmd

NC = 8           # cores
P = 128          # partitions
ROWW = 128       # bf16 feature-table row width (256 bytes)
PHROWS = 32768   # gather index range per phase (int16 limit)
CHUNK = 4096     # edge positions per gather call (needs 64K SWDGE scratch + multi-packet)
WIDE = 1024      # x-tile width for layer-1 matmul loads

BF16 = mybir.dt.bfloat16
F32 = mybir.dt.float32
I16 = mybir.dt.int16

DEBUG_SKIP_EDGE = False   # skip gather/matmul edge phases (crash bisect)
DEBUG_LOCAL_AG = False    # replace AllGather with local copy (crash bisect)
NSL = 8          # AllGather slices (one >~3MB collective wedges the device)
DMA_SCRATCH = 65536   # SWDGE descriptor-ring carveout (bytes)
NSWQ = 4              # SWDGE queues: queue q desc-gens on Q7 cores 2q,2q+1 (parallel)
SINGLE_PACKET = False  # multi-packet: one call's descs may exceed one ring packet


# ----------------------------------------------------------------- host prep

def _host_prep(x, edge_index, W1, b1, W2, b2):
    N, IN_DIM = x.shape
    HID = W1.shape[1]
    OUT = W2.shape[1]
    assert N % NC == 0
    SH = N // NC                      # real rows per shard
    SHP = -(-SH // P) * P             # padded rows per shard
    NBLK = SHP // P
    R = NC * SHP                      # padded table rows
    NPH = -(-R // PHROWS)

    src = np.asarray(edge_index[0], dtype=np.int64)
    dst = np.asarray(edge_index[1], dtype=np.int64)
    E = src.shape[0]

    deg = np.bincount(dst, minlength=N).astype(np.float64) + 1.0
    dinv = (1.0 / np.sqrt(deg)).astype(np.float32)

    # table row for each source node. The table is written by NSL sliced
    # AllGathers with contiguous outputs, so its layout is
    # [slice, core, rows-in-slice]: row = i*NC*SL + c*SL + r.
    SL = SHP // NSL
    assert SHP % NSL == 0
    sc = src // SH
    sa = src % SH
    srow = (sa // SL) * (NC * SL) + sc * SL + (sa % SL)
    phase = srow // PHROWS
    lidx = (srow % PHROWS).astype(np.int16)
    core = dst // SH
    blk = (dst % SH) // P
    dlo = ((dst % SH) % P).astype(np.int16)

    # group edges per (core, phase, block)
    order = np.lexsort((blk, phase, core))
    src_s, lidx_s, phase_s, core_s, blk_s, dlo_s = (
        src[order], lidx[order], phase[order], core[order], blk[order], dlo[order])

    # counts per (core, phase, block)
    key = (core_s * NPH + phase_s) * NBLK + blk_s
    cnt = np.bincount(key, minlength=NC * NPH * NBLK).reshape(NC, NPH, NBLK)
    Gsb = -(-cnt.max(axis=0) // P)            # [NPH, NBLK] groups, shared

    seg_pos = {}                              # (s, b) -> start position
    posn = 0
    segments = []                             # (s, b, ngroups) in emission order
    for s in range(NPH):
        for b in range(NBLK):
            g = int(Gsb[s, b])
            if g == 0:
                continue
            seg_pos[(s, b)] = posn
            segments.append((s, b, g))
            posn += g * P
    TTOT = posn
    assert TTOT % P == 0

    # chunks: pack segment pieces of one phase, <= CHUNK positions each.
    # A chunk's segs list holds (b, ngroups, first_piece, last_piece);
    # PSUM accumulation runs may span chunks (first/last flags drive
    # start= and the final accumulate into acc).
    chunks = []                               # (s, pos0, npos, [(b, ng, fst, lst)])
    cur = None
    for (s, b, g) in segments:
        gleft = g
        first = True
        while gleft > 0:
            if cur is not None and (cur[0] != s or cur[2] >= CHUNK):
                chunks.append(cur)
                cur = None
            if cur is None:
                cur = [s, seg_pos[(s, b)] + (g - gleft) * P, 0, []]
            take = min(gleft, (CHUNK - cur[2]) // P)
            cur[2] += take * P
            gleft -= take
            cur[3].append((b, take, first, gleft == 0))
            first = False
    if cur is not None:
        chunks.append(cur)

    # per-core position-indexed arrays
    starts = np.zeros(NC, np.int64)
    idx_all = np.zeros((NC, TTOT), np.int16)
    dlo_all = np.full((NC, TTOT), -1.0, np.float32)
    # edges of (c,s,b) occupy positions seg_pos[(s,b)] .. +cnt[c,s,b]
    csb_off = np.zeros(NC * NPH * NBLK + 1, np.int64)
    np.cumsum(cnt.reshape(-1), out=csb_off[1:])
    for c in range(NC):
        for s in range(NPH):
            for b in range(NBLK):
                n = int(cnt[c, s, b])
                if n == 0:
                    continue
                o = int(csb_off[(c * NPH + s) * NBLK + b])
                p0 = seg_pos[(s, b)]
                idx_all[c, p0:p0 + n] = lidx_s[o:o + n]
                dlo_all[c, p0:p0 + n] = dlo_s[o:o + n]
    del starts

    # wrap gather indices: [128, TTOT/16], 16-partition wrap replicated x8
    idx_w = np.ascontiguousarray(
        np.tile(idx_all.reshape(NC, TTOT // 16, 16).transpose(0, 2, 1), (1, 8, 1)))
    # dst values transposed: [128, TTOT/128]
    dst_t = np.ascontiguousarray(
        dlo_all.reshape(NC, TTOT // P, P).transpose(0, 2, 1)).astype(ml_dtypes.bfloat16)

    # x transposed & padded per core: [KCH, 128, SHP] bf16
    KCH = IN_DIM // P
    xtp = np.zeros((NC, KCH, P, SHP), ml_dtypes.bfloat16)
    xs = x.reshape(NC, SH, IN_DIM).astype(ml_dtypes.bfloat16)
    xtp[:, :, :, :SH] = xs.transpose(0, 2, 1).reshape(NC, KCH, P, SH)

    # W1 packed [128, KCH*HID] bf16
    w1p = np.ascontiguousarray(
        W1.reshape(KCH, P, HID).transpose(1, 0, 2).reshape(P, KCH * HID)
    ).astype(ml_dtypes.bfloat16)
    w2p = np.asarray(W2, np.float32)                       # [HID, OUT]

    dinv_pad = np.zeros((NC, SHP), np.float32)
    dinv_pad[:, :SH] = dinv.reshape(NC, SH)
    dinvc = np.ascontiguousarray(
        dinv_pad.reshape(NC, NBLK, P).transpose(0, 2, 1))  # [NC, 128, NBLK]

    b1r = np.tile(np.asarray(b1, np.float32)[None, :], (P, 1))
    b2r = np.tile(np.asarray(b2, np.float32)[None, :], (P, 1))
    iota = np.tile(np.arange(P, dtype=np.float32)[None, :], (P, CHUNK // P)
                   ).astype(ml_dtypes.bfloat16)
    ident = np.eye(P, dtype=np.float32)

    meta = dict(N=N, IN_DIM=IN_DIM, HID=HID, OUT=OUT, SH=SH, SHP=SHP,
                NBLK=NBLK, R=R, NPH=NPH, KCH=KCH, TTOT=TTOT, chunks=chunks)
    in_maps = []
    for c in range(NC):
        in_maps.append({
            "xt": np.ascontiguousarray(xtp[c]),
            "w1": w1p,
            "w2": w2p,
            "dinvc": np.ascontiguousarray(dinvc[c]),
            "b1r": b1r,
            "b2r": b2r,
            "iota": iota,
            "ident": ident,
            "gidx": idx_w[c],
            "dstv": dst_t[c],
        })
    return in_maps, meta


# ------------------------------------------------------------- device program

def _emit_edge_phase(nc, tc, stack_pools, meta, g_full, acc_ap, F,
                     dst_sb, iota_sb, gidx_dram, layer):
    """Gather + one-hot matmul accumulate for one layer. acc_ap: [128, NBLK*F]."""
    if DEBUG_SKIP_EDGE:
        return
    chunks = meta["chunks"]
    R = meta["R"]
    idxp, msgp, sp, psp = stack_pools
    open_ps = {}   # b -> (psum tile, n groups so far)
    for ci, (s, pos0, npos, segs) in enumerate(chunks):
        row0 = s * PHROWS
        row1 = min(row0 + PHROWS, R)
        idx_t = idxp.tile([P, npos // 16], I16, name=f"idx{layer}_{ci}", tag="idx")
        nc.sync.dma_start(idx_t[:], gidx_dram[:, pos0 // 16:(pos0 + npos) // 16])
        msgs = msgp.tile([P, npos // P, ROWW], BF16, name=f"msg{layer}_{ci}", tag="msgs")
        nc.gpsimd.dma_gather(
            out_ap=msgs[:],
            in_ap=g_full[row0:row1, :],
            idxs_ap=idx_t[:],
            num_idxs=npos,
            num_idxs_reg=npos,
            elem_size=ROWW,
            single_packet=SINGLE_PACKET,
            queue_num=ci % NSWQ,
        )
        S = sp.tile([P, npos], BF16, name=f"S{layer}_{ci}", tag="S")
        ngr = npos // P
        nc.vector.tensor_tensor(
            out=S[:].rearrange("p (g j) -> p g j", j=P),
            in0=dst_sb[:, pos0 // P:pos0 // P + ngr].to_broadcast([P, ngr, P]),
            in1=iota_sb[:, :npos].rearrange("p (g j) -> p g j", j=P),
            op=mybir.AluOpType.is_equal,
        )
        g = 0
        for (b, ng, fst, lst) in segs:
            if fst:
                ps = psp.tile([P, F], F32, space="PSUM",
                              name=f"ps{layer}_{ci}_{b}", tag="ps")
                done = 0
            else:
                ps, done = open_ps.pop(b)
            for i in range(ng):
                nc.tensor.matmul(
                    ps[:],
                    lhsT=S[:, (g + i) * P:(g + i + 1) * P],
                    rhs=msgs[:, g + i, :F],
                    start=(done + i == 0),
                    stop=(lst and i == ng - 1),
                )
            if lst:
                nc.vector.tensor_add(
                    acc_ap[:, b * F:(b + 1) * F],
                    acc_ap[:, b * F:(b + 1) * F], ps[:])
            else:
                open_ps[b] = (ps, done + ng)
            g += ng


def _build_program(meta):
    N, HID, OUT = meta["N"], meta["HID"], meta["OUT"]
    SHP, NBLK, R, KCH, TTOT = (meta["SHP"], meta["NBLK"], meta["R"],
                               meta["KCH"], meta["TTOT"])

    nc = bacc.Bacc("TRN2", target_bir_lowering=False, debug=False, num_devices=NC,
                   dynamic_dma_scratch_size=DMA_SCRATCH, num_swdge_queues=NSWQ)

    t_xt = nc.dram_tensor("xt", [KCH, P, SHP], BF16, kind="ExternalInput")
    t_w1 = nc.dram_tensor("w1", [P, KCH * HID], BF16, kind="ExternalInput")
    t_w2 = nc.dram_tensor("w2", [HID, OUT], F32, kind="ExternalInput")
    t_dinvc = nc.dram_tensor("dinvc", [P, NBLK], F32, kind="ExternalInput")
    t_b1r = nc.dram_tensor("b1r", [P, HID], F32, kind="ExternalInput")
    t_b2r = nc.dram_tensor("b2r", [P, OUT], F32, kind="ExternalInput")
    t_iota = nc.dram_tensor("iota", [P, CHUNK], BF16, kind="ExternalInput")
    t_ident = nc.dram_tensor("ident", [P, P], F32, kind="ExternalInput")
    t_gidx = nc.dram_tensor("gidx", [P, TTOT // 16], I16, kind="ExternalInput")
    t_dstv = nc.dram_tensor("dstv", [P, TTOT // P], BF16, kind="ExternalInput")
    t_out = nc.dram_tensor("out", [SHP, OUT], F32, kind="ExternalOutput")

    g1_c = nc.dram_tensor("g1_c", [SHP, ROWW], BF16)
    g1_full = nc.dram_tensor("g1_full", [R, ROWW], BF16, addr_space="Shared")
    g2_c = nc.dram_tensor("g2_c", [SHP, ROWW], BF16)
    g2_full = nc.dram_tensor("g2_full", [R, ROWW], BF16, addr_space="Shared")
    o1t_d = nc.dram_tensor("o1t_d", [HID, SHP], F32)

    with tile.TileContext(nc) as tc:
        with tc.tile_pool(name="persist", bufs=1) as pers:
            w1_sb = pers.tile([P, KCH * HID], BF16)
            nc.sync.dma_start(w1_sb[:], t_w1[:])
            w2_sb = pers.tile([HID, OUT], F32)
            nc.sync.dma_start(w2_sb[:], t_w2[:])
            dinv_sb = pers.tile([P, NBLK], F32)
            nc.sync.dma_start(dinv_sb[:], t_dinvc[:])
            b1_sb = pers.tile([P, HID], F32)
            nc.sync.dma_start(b1_sb[:], t_b1r[:])
            b2_sb = pers.tile([P, OUT], F32)
            nc.sync.dma_start(b2_sb[:], t_b2r[:])
            iota_sb = pers.tile([P, CHUNK], BF16)
            nc.sync.dma_start(iota_sb[:], t_iota[:])
            ident_sb = pers.tile([P, P], F32)
            nc.sync.dma_start(ident_sb[:], t_ident[:])
            dst_sb = pers.tile([P, TTOT // P], BF16)
            nc.sync.dma_start(dst_sb[:], t_dstv[:])

            # ======== layer 1 scope (acc1/g1loc live M1 .. transpose) ========
            with tc.tile_pool(name="l1", bufs=1) as l1p:
                g1loc = l1p.tile([P, NBLK * HID], F32)
                acc1 = l1p.tile([P, NBLK * HID], F32)
                nc.vector.memset(acc1[:], 0.0)

                # ---- layer 1 matmul:  g1 = dinv * (x @ W1)
                with (tc.tile_pool(name="m1x", bufs=3) as xp,
                      tc.tile_pool(name="m1ps", bufs=4, space="PSUM") as m1psp,
                      tc.tile_pool(name="m1o", bufs=3) as m1op):
                    nwide = -(-SHP // WIDE)
                    for wi in range(nwide):
                        c0 = wi * WIDE
                        ncols = min(WIDE, SHP - c0)
                        xw = []
                        for k in range(KCH):
                            xt_k = xp.tile([P, ncols], BF16,
                                           name=f"xw{wi}_{k}", tag=f"xw{k}")
                            nc.sync.dma_start(xt_k[:], t_xt[k, :, c0:c0 + ncols])
                            xw.append(xt_k)
                        for rb in range(ncols // P):
                            gb = c0 // P + rb
                            ps = m1psp.tile([P, HID], F32, space="PSUM",
                                            name=f"m1ps{gb}", tag="m1ps")
                            for k in range(KCH):
                                nc.tensor.matmul(
                                    ps[:],
                                    lhsT=xw[k][:, rb * P:(rb + 1) * P],
                                    rhs=w1_sb[:, k * HID:(k + 1) * HID],
                                    start=(k == 0),
                                    stop=(k == KCH - 1),
                                )
                            nc.vector.tensor_scalar_mul(
                                g1loc[:, gb * HID:(gb + 1) * HID], ps[:],
                                dinv_sb[:, gb:gb + 1])
                            g1b = m1op.tile([P, HID], BF16,
                                            name=f"g1b{gb}", tag="g1b")
                            nc.vector.tensor_copy(
                                g1b[:], g1loc[:, gb * HID:(gb + 1) * HID])
                            nc.sync.dma_start(
                                g1_c[gb * P:(gb + 1) * P, 0:HID], g1b[:])

                # ---- AllGather layer-1 table (sliced; table layout
                # [slice, core, rows] so each collective output is contiguous)
                SL = SHP // NSL
                if DEBUG_LOCAL_AG:
                    nc.sync.dma_start(g1_full[0:SHP, :], g1_c[:])
                else:
                    for i in range(NSL):
                        nc.gpsimd.collective_compute(
                            "AllGather", mybir.AluOpType.bypass,
                            replica_groups=[list(range(NC))],
                            ins=[g1_c[i * SL:(i + 1) * SL, :]],
                            outs=[g1_full[i * NC * SL:(i + 1) * NC * SL, :]],
                        )

                # ---- layer 1 edge phase
                with (tc.tile_pool(name="e1idx", bufs=3) as idxp,
                      tc.tile_pool(name="e1msg", bufs=2) as msgp,
                      tc.tile_pool(name="e1S", bufs=2) as sp,
                      tc.tile_pool(name="e1ps", bufs=4, space="PSUM") as psp):
                    _emit_edge_phase(nc, tc, (idxp, msgp, sp, psp), meta,
                                     g1_full, acc1[:], HID, dst_sb, iota_sb,
                                     t_gidx, 1)

                # ---- layer-1 epilogue: out1 = relu(dinv*(acc+g1loc)+b1)
                a3 = acc1[:].rearrange("p (n h) -> p n h", h=HID)
                nc.vector.tensor_add(acc1[:], acc1[:], g1loc[:])
                nc.vector.tensor_tensor(
                    out=a3, in0=a3, in1=dinv_sb[:].to_broadcast([P, NBLK, HID]),
                    op=mybir.AluOpType.mult)
                nc.vector.tensor_tensor(
                    out=a3, in0=a3,
                    in1=b1_sb[:].to_broadcast([P, HID, NBLK]
                                              ).rearrange("p h n -> p n h"),
                    op=mybir.AluOpType.add)
                nc.vector.tensor_scalar_max(acc1[:], acc1[:], 0.0)

                # ---- transpose out1 -> o1t_d DRAM [HID, SHP]
                with (tc.tile_pool(name="tp", bufs=4, space="PSUM") as tpp,
                      tc.tile_pool(name="tpo", bufs=3) as tpo):
                    for gb in range(NBLK):
                        pst = tpp.tile([HID, P], F32, space="PSUM",
                                       name=f"pst{gb}", tag="pst")
                        nc.tensor.transpose(
                            pst[:], acc1[:, gb * HID:(gb + 1) * HID], ident_sb[:])
                        o1s = tpo.tile([HID, P], F32, name=f"o1s{gb}", tag="o1s")
                        nc.vector.tensor_copy(o1s[:], pst[:])
                        nc.sync.dma_start(o1t_d[:, gb * P:(gb + 1) * P], o1s[:])

            # ======== layer 2 scope ========
            with tc.tile_pool(name="l2", bufs=1) as l2p:
                g2loc = l2p.tile([P, NBLK * OUT], F32)
                acc2 = l2p.tile([P, NBLK * OUT], F32)
                nc.vector.memset(acc2[:], 0.0)

                # ---- layer-2 matmul: g2 = dinv * (out1 @ W2)
                with (tc.tile_pool(name="m2x", bufs=3) as o1xp,
                      tc.tile_pool(name="m2ps", bufs=4, space="PSUM") as m2psp,
                      tc.tile_pool(name="m2o", bufs=3) as m2op):
                    nwide = -(-SHP // WIDE)
                    for wi in range(nwide):
                        c0 = wi * WIDE
                        ncols = min(WIDE, SHP - c0)
                        o1w = o1xp.tile([HID, ncols], F32,
                                        name=f"o1w{wi}", tag="o1w")
                        nc.sync.dma_start(o1w[:], o1t_d[:, c0:c0 + ncols])
                        for rb in range(ncols // P):
                            gb = c0 // P + rb
                            ps2 = m2psp.tile([P, OUT], F32, space="PSUM",
                                             name=f"m2ps{gb}", tag="m2ps")
                            nc.tensor.matmul(
                                ps2[:], lhsT=o1w[:, rb * P:(rb + 1) * P],
                                rhs=w2_sb[:], start=True, stop=True)
                            nc.vector.tensor_scalar_mul(
                                g2loc[:, gb * OUT:(gb + 1) * OUT], ps2[:],
                                dinv_sb[:, gb:gb + 1])
                            g2b = m2op.tile([P, OUT], BF16,
                                            name=f"g2b{gb}", tag="g2b")
                            nc.vector.tensor_copy(
                                g2b[:], g2loc[:, gb * OUT:(gb + 1) * OUT])
                            nc.sync.dma_start(
                                g2_c[gb * P:(gb + 1) * P, 0:OUT], g2b[:])

                SL = SHP // NSL
                if DEBUG_LOCAL_AG:
                    nc.sync.dma_start(g2_full[0:SHP, :], g2_c[:])
                else:
                    for i in range(NSL):
                        nc.gpsimd.collective_compute(
                            "AllGather", mybir.AluOpType.bypass,
                            replica_groups=[list(range(NC))],
                            ins=[g2_c[i * SL:(i + 1) * SL, :]],
                            outs=[g2_full[i * NC * SL:(i + 1) * NC * SL, :]],
                        )

                # ---- layer 2 edge phase
                with (tc.tile_pool(name="e2idx", bufs=3) as idxp,
                      tc.tile_pool(name="e2msg", bufs=2) as msgp,
                      tc.tile_pool(name="e2S", bufs=2) as sp,
                      tc.tile_pool(name="e2ps", bufs=4, space="PSUM") as psp):
                    _emit_edge_phase(nc, tc, (idxp, msgp, sp, psp), meta,
                                     g2_full, acc2[:], OUT, dst_sb, iota_sb,
                                     t_gidx, 2)

                # ---- layer-2 epilogue: out = dinv*(acc2+g2loc)+b2
                c3 = acc2[:].rearrange("p (n h) -> p n h", h=OUT)
                nc.vector.tensor_add(acc2[:], acc2[:], g2loc[:])
                nc.vector.tensor_tensor(
                    out=c3, in0=c3, in1=dinv_sb[:].to_broadcast([P, NBLK, OUT]),
                    op=mybir.AluOpType.mult)
                nc.vector.tensor_tensor(
                    out=c3, in0=c3,
                    in1=b2_sb[:].to_broadcast([P, OUT, NBLK]
                                              ).rearrange("p h n -> p n h"),
                    op=mybir.AluOpType.add)
                for gb in range(NBLK):
                    nc.sync.dma_start(
                        t_out[gb * P:(gb + 1) * P, :],
                        acc2[:, gb * OUT:(gb + 1) * OUT])

    nc.compile()
    return nc


# ------------------------------------------------------------------ frontend

_CACHE = {}


def run(trace=False, **inputs):
    in_maps, meta = _host_prep(
        inputs["x"], inputs["edge_index"], inputs["W1"], inputs["b1"],
        inputs["W2"], inputs["b2"])
    key = (meta["N"], meta["IN_DIM"], meta["HID"], meta["OUT"], meta["TTOT"],
           tuple((s, p, n, tuple(sg)) for s, p, n, sg in meta["chunks"]))
    if key not in _CACHE:
        _CACHE.clear()
        _CACHE[key] = _build_program(meta)
    nc = _CACHE[key]
    res = run_bass_kernel_spmd(nc, in_maps, list(range(NC)), trace=trace)
    SH = meta["SH"]
    out = np.concatenate([res.results[c]["out"][:SH] for c in range(NC)], axis=0)
    return out.astype(np.float32), res


def kernel(**inputs):
    out, _ = run(trace=False, **inputs)
    return out


# revision 17
# speedup vs baseline: 2.0873x; 1.5249x over previous
"""Two-layer GCN forward on 8 Trainium2 NeuronCores (Bass/Tile).

Strategy (graph/data parallel, dst-sharded):
  - Nodes sharded across 8 cores (12500/core, padded to 12544 = 98*128).
  - Per layer: sharded matmul h = x @ W, pre-scaled g = dinv * h, cast bf16,
    AllGather the per-node feature table to every core (256B rows).
  - Each core owns the edges whose dst lies in its shard. Per-edge work:
    dma_gather of g[src] rows (256B HBM reads) -> SBUF messages; a one-hot
    "selection" matrix built on the vector engine (dst_local == iota) turns
    the scatter-add into PE matmuls accumulating per-128-node dst block in
    PSUM.  out = relu(dinv*(acc + g_local) + b).
  - Self-loop term folded into the epilogue (g_local), never gathered.
  - Normalization dinv[src]*dinv[dst] is folded into per-node pre/post
    scaling, so the per-edge path is pure gather + matmul.

Edge indices are int16-limited (32768 rows/call), so the gathered table is
processed in 4 row-phases; host buckets each core's edges by
(phase, dst-block) with per-(phase,block) segment sizes padded to 128 and
shared across cores (SPMD: one program, per-core data).
"""

import numpy as np
import ml_dtypes

import concourse.bass as bass
import concourse.tile as tile
from concourse import bacc, mybir
from concourse.bass_utils import run_bass_kernel_spmd

NC = 8           # cores
P = 128          # partitions
ROWW = 128       # bf16 feature-table row width (256 bytes)
PHROWS = 32768   # gather index range per phase (int16 limit)
CHUNK = 4096     # edge positions per gather call (needs 64K SWDGE scratch + multi-packet)
WIDE = 1024      # x-tile width for layer-1 matmul loads

BF16 = mybir.dt.bfloat16
F32 = mybir.dt.float32
I16 = mybir.dt.int16

DEBUG_SKIP_EDGE = False   # skip gather/matmul edge phases (crash bisect)
DEBUG_LOCAL_AG = False    # replace AllGather with local copy (crash bisect)
NSL = 8          # AllGather slices (one >~3MB collective wedges the device)
DMA_SCRATCH = 65536   # SWDGE descriptor-ring carveout (bytes)
NSWQ = 4              # SWDGE queues: queue q desc-gens on Q7 cores 2q,2q+1 (parallel)
SINGLE_PACKET = False  # multi-packet: one call's descs may exceed one ring packet


# ----------------------------------------------------------------- host prep

def _host_prep(x, edge_index, W1, b1, W2, b2):
    N, IN_DIM = x.shape
    HID = W1.shape[1]
    OUT = W2.shape[1]
    assert N % NC == 0
    SH = N // NC                      # real rows per shard
    SHP = -(-SH // P) * P             # padded rows per shard
    NBLK = SHP // P
    R = NC * SHP                      # padded table rows
    NPH = -(-R // PHROWS)

    src = np.asarray(edge_index[0], dtype=np.int64)
    dst = np.asarray(edge_index[1], dtype=np.int64)
    E = src.shape[0]

    deg = np.bincount(dst, minlength=N).astype(np.float64) + 1.0
    dinv = (1.0 / np.sqrt(deg)).astype(np.float32)

    # table row for each source node. The table is written by NSL sliced
    # AllGathers with contiguous outputs, so its layout is
    # [slice, core, rows-in-slice]: row = i*NC*SL + c*SL + r.
    SL = SHP // NSL
    assert SHP % NSL == 0
    sc = src // SH
    sa = src % SH
    srow = (sa // SL) * (NC * SL) + sc * SL + (sa % SL)
    phase = srow // PHROWS
    lidx = (srow % PHROWS).astype(np.int16)
    core = dst // SH
    blk = (dst % SH) // P
    dlo = ((dst % SH) % P).astype(np.int16)

    # group edges per (core, phase, block)
    order = np.lexsort((blk, phase, core))
    src_s, lidx_s, phase_s, core_s, blk_s, dlo_s = (
        src[order], lidx[order], phase[order], core[order], blk[order], dlo[order])

    # counts per (core, phase, block)
    key = (core_s * NPH + phase_s) * NBLK + blk_s
    cnt = np.bincount(key, minlength=NC * NPH * NBLK).reshape(NC, NPH, NBLK)
    Gsb = -(-cnt.max(axis=0) // P)            # [NPH, NBLK] groups, shared

    seg_pos = {}                              # (s, b) -> start position
    posn = 0
    segments = []                             # (s, b, ngroups) in emission order
    for s in range(NPH):
        for b in range(NBLK):
            g = int(Gsb[s, b])
            if g == 0:
                continue
            seg_pos[(s, b)] = posn
            segments.append((s, b, g))
            posn += g * P
    TTOT = posn
    assert TTOT % P == 0

    # chunks: pack segment pieces of one phase, <= CHUNK positions each.
    # A chunk's segs list holds (b, ngroups, first_piece, last_piece);
    # PSUM accumulation runs may span chunks (first/last flags drive
    # start= and the final accumulate into acc).
    chunks = []                               # (s, pos0, npos, [(b, ng, fst, lst)])
    cur = None
    for (s, b, g) in segments:
        gleft = g
        first = True
        while gleft > 0:
            if cur is not None and (cur[0] != s or cur[2] >= CHUNK):
                chunks.append(cur)
                cur = None
            if cur is None:
                cur = [s, seg_pos[(s, b)] + (g - gleft) * P, 0, []]
            take = min(gleft, (CHUNK - cur[2]) // P)
            cur[2] += take * P
            gleft -= take
            cur[3].append((b, take, first, gleft == 0))
            first = False
    if cur is not None:
        chunks.append(cur)

    # per-core position-indexed arrays
    starts = np.zeros(NC, np.int64)
    idx_all = np.zeros((NC, TTOT), np.int16)
    dlo_all = np.full((NC, TTOT), -1.0, np.float32)
    # edges of (c,s,b) occupy positions seg_pos[(s,b)] .. +cnt[c,s,b]
    csb_off = np.zeros(NC * NPH * NBLK + 1, np.int64)
    np.cumsum(cnt.reshape(-1), out=csb_off[1:])
    for c in range(NC):
        for s in range(NPH):
            for b in range(NBLK):
                n = int(cnt[c, s, b])
                if n == 0:
                    continue
                o = int(csb_off[(c * NPH + s) * NBLK + b])
                p0 = seg_pos[(s, b)]
                idx_all[c, p0:p0 + n] = lidx_s[o:o + n]
                dlo_all[c, p0:p0 + n] = dlo_s[o:o + n]
    del starts

    # wrap gather indices: [128, TTOT/16], 16-partition wrap replicated x8
    idx_w = np.ascontiguousarray(
        np.tile(idx_all.reshape(NC, TTOT // 16, 16).transpose(0, 2, 1), (1, 8, 1)))
    # dst values transposed: [128, TTOT/128]
    dst_t = np.ascontiguousarray(
        dlo_all.reshape(NC, TTOT // P, P).transpose(0, 2, 1)).astype(ml_dtypes.bfloat16)

    # x transposed & padded per core: [KCH, 128, SHP] bf16
    KCH = IN_DIM // P
    xtp = np.zeros((NC, KCH, P, SHP), ml_dtypes.bfloat16)
    xs = x.reshape(NC, SH, IN_DIM).astype(ml_dtypes.bfloat16)
    xtp[:, :, :, :SH] = xs.transpose(0, 2, 1).reshape(NC, KCH, P, SH)

    # W1 packed [128, KCH*HID] bf16
    w1p = np.ascontiguousarray(
        W1.reshape(KCH, P, HID).transpose(1, 0, 2).reshape(P, KCH * HID)
    ).astype(ml_dtypes.bfloat16)
    w2p = np.asarray(W2, np.float32)                       # [HID, OUT]

    dinv_pad = np.zeros((NC, SHP), np.float32)
    dinv_pad[:, :SH] = dinv.reshape(NC, SH)
    dinvc = np.ascontiguousarray(
        dinv_pad.reshape(NC, NBLK, P).transpose(0, 2, 1))  # [NC, 128, NBLK]

    b1r = np.tile(np.asarray(b1, np.float32)[None, :], (P, 1))
    b2r = np.tile(np.asarray(b2, np.float32)[None, :], (P, 1))
    iota = np.tile(np.arange(P, dtype=np.float32)[None, :], (P, CHUNK // P)
                   ).astype(ml_dtypes.bfloat16)
    ident = np.eye(P, dtype=np.float32)

    meta = dict(N=N, IN_DIM=IN_DIM, HID=HID, OUT=OUT, SH=SH, SHP=SHP,
                NBLK=NBLK, R=R, NPH=NPH, KCH=KCH, TTOT=TTOT, chunks=chunks)
    in_maps = []
    for c in range(NC):
        in_maps.append({
            "xt": np.ascontiguousarray(xtp[c]),
            "w1": w1p,
            "w2": w2p,
            "dinvc": np.ascontiguousarray(dinvc[c]),
            "b1r": b1r,
            "b2r": b2r,
            "iota": iota,
            "ident": ident,
            "gidx": idx_w[c],
            "dstv": dst_t[c],
        })
    return in_maps, meta


# ------------------------------------------------------------- device program

def _emit_edge_phase(nc, tc, stack_pools, meta, g_full, acc_ap, F,
                     dst_sb, iota_sb, gidx_dram, layer):
    """Gather + one-hot matmul accumulate for one layer. acc_ap: [128, NBLK*F]."""
    if DEBUG_SKIP_EDGE:
        return
    chunks = meta["chunks"]
    R = meta["R"]
    idxp, msgp, sp, psp = stack_pools
    open_ps = {}   # b -> (psum tile, n groups so far)
    for ci, (s, pos0, npos, segs) in enumerate(chunks):
        row0 = s * PHROWS
        row1 = min(row0 + PHROWS, R)
        idx_t = idxp.tile([P, npos // 16], I16, name=f"idx{layer}_{ci}", tag="idx")
        nc.sync.dma_start(idx_t[:], gidx_dram[:, pos0 // 16:(pos0 + npos) // 16])
        msgs = msgp.tile([P, npos // P, ROWW], BF16, name=f"msg{layer}_{ci}", tag="msgs")
        nc.gpsimd.dma_gather(
            out_ap=msgs[:],
            in_ap=g_full[row0:row1, :],
            idxs_ap=idx_t[:],
            num_idxs=npos,
            num_idxs_reg=npos,
            elem_size=ROWW,
            single_packet=SINGLE_PACKET,
            queue_num=ci % NSWQ,
        )
        S = sp.tile([P, npos], BF16, name=f"S{layer}_{ci}", tag="S")
        ngr = npos // P
        nc.vector.tensor_tensor(
            out=S[:].rearrange("p (g j) -> p g j", j=P),
            in0=dst_sb[:, pos0 // P:pos0 // P + ngr].to_broadcast([P, ngr, P]),
            in1=iota_sb[:, :npos].rearrange("p (g j) -> p g j", j=P),
            op=mybir.AluOpType.is_equal,
        )
        g = 0
        for (b, ng, fst, lst) in segs:
            if fst:
                ps = psp.tile([P, F], F32, space="PSUM",
                              name=f"ps{layer}_{ci}_{b}", tag="ps")
                done = 0
            else:
                ps, done = open_ps.pop(b)
            for i in range(ng):
                nc.tensor.matmul(
                    ps[:],
                    lhsT=S[:, (g + i) * P:(g + i + 1) * P],
                    rhs=msgs[:, g + i, :F],
                    start=(done + i == 0),
                    stop=(lst and i == ng - 1),
                )
            if lst:
                nc.vector.tensor_add(
                    acc_ap[:, b * F:(b + 1) * F],
                    acc_ap[:, b * F:(b + 1) * F], ps[:])
            else:
                open_ps[b] = (ps, done + ng)
            g += ng


def _build_program(meta):
    N, HID, OUT = meta["N"], meta["HID"], meta["OUT"]
    SHP, NBLK, R, KCH, TTOT = (meta["SHP"], meta["NBLK"], meta["R"],
                               meta["KCH"], meta["TTOT"])

    nc = bacc.Bacc("TRN2", target_bir_lowering=False, debug=False, num_devices=NC,
                   dynamic_dma_scratch_size=DMA_SCRATCH, num_swdge_queues=NSWQ)

    t_xt = nc.dram_tensor("xt", [KCH, P, SHP], BF16, kind="ExternalInput")
    t_w1 = nc.dram_tensor("w1", [P, KCH * HID], BF16, kind="ExternalInput")
    t_w2 = nc.dram_tensor("w2", [HID, OUT], F32, kind="ExternalInput")
    t_dinvc = nc.dram_tensor("dinvc", [P, NBLK], F32, kind="ExternalInput")
    t_b1r = nc.dram_tensor("b1r", [P, HID], F32, kind="ExternalInput")
    t_b2r = nc.dram_tensor("b2r", [P, OUT], F32, kind="ExternalInput")
    t_iota = nc.dram_tensor("iota", [P, CHUNK], BF16, kind="ExternalInput")
    t_ident = nc.dram_tensor("ident", [P, P], F32, kind="ExternalInput")
    t_gidx = nc.dram_tensor("gidx", [P, TTOT // 16], I16, kind="ExternalInput")
    t_dstv = nc.dram_tensor("dstv", [P, TTOT // P], BF16, kind="ExternalInput")
    t_out = nc.dram_tensor("out", [SHP, OUT], F32, kind="ExternalOutput")

    g1_c = nc.dram_tensor("g1_c", [SHP, ROWW], BF16)
    g1_full = nc.dram_tensor("g1_full", [R, ROWW], BF16, addr_space="Shared")
    g2_c = nc.dram_tensor("g2_c", [SHP, ROWW], BF16)
    g2_full = nc.dram_tensor("g2_full", [R, ROWW], BF16, addr_space="Shared")
    o1t_d = nc.dram_tensor("o1t_d", [HID, SHP], F32)

    with tile.TileContext(nc) as tc:
        with tc.tile_pool(name="persist", bufs=1) as pers:
            w1_sb = pers.tile([P, KCH * HID], BF16)
            nc.sync.dma_start(w1_sb[:], t_w1[:])
            w2_sb = pers.tile([HID, OUT], F32)
            nc.sync.dma_start(w2_sb[:], t_w2[:])
            dinv_sb = pers.tile([P, NBLK], F32)
            nc.sync.dma_start(dinv_sb[:], t_dinvc[:])
            b1_sb = pers.tile([P, HID], F32)
            nc.sync.dma_start(b1_sb[:], t_b1r[:])
            b2_sb = pers.tile([P, OUT], F32)
            nc.sync.dma_start(b2_sb[:], t_b2r[:])
            iota_sb = pers.tile([P, CHUNK], BF16)
            nc.sync.dma_start(iota_sb[:], t_iota[:])
            ident_sb = pers.tile([P, P], F32)
            nc.sync.dma_start(ident_sb[:], t_ident[:])
            dst_sb = pers.tile([P, TTOT // P], BF16)
            nc.sync.dma_start(dst_sb[:], t_dstv[:])

            # ======== layer 1 scope (acc1/g1loc live M1 .. transpose) ========
            with tc.tile_pool(name="l1", bufs=1) as l1p:
                g1loc = l1p.tile([P, NBLK * HID], F32)
                acc1 = l1p.tile([P, NBLK * HID], F32)
                nc.vector.memset(acc1[:], 0.0)

                # ---- layer 1 matmul:  g1 = dinv * (x @ W1)
                with (tc.tile_pool(name="m1x", bufs=3) as xp,
                      tc.tile_pool(name="m1ps", bufs=4, space="PSUM") as m1psp,
                      tc.tile_pool(name="m1o", bufs=3) as m1op):
                    nwide = -(-SHP // WIDE)
                    for wi in range(nwide):
                        c0 = wi * WIDE
                        ncols = min(WIDE, SHP - c0)
                        xw = []
                        for k in range(KCH):
                            xt_k = xp.tile([P, ncols], BF16,
                                           name=f"xw{wi}_{k}", tag=f"xw{k}")
                            nc.sync.dma_start(xt_k[:], t_xt[k, :, c0:c0 + ncols])
                            xw.append(xt_k)
                        for rb in range(ncols // P):
                            gb = c0 // P + rb
                            ps = m1psp.tile([P, HID], F32, space="PSUM",
                                            name=f"m1ps{gb}", tag="m1ps")
                            for k in range(KCH):
                                nc.tensor.matmul(
                                    ps[:],
                                    lhsT=xw[k][:, rb * P:(rb + 1) * P],
                                    rhs=w1_sb[:, k * HID:(k + 1) * HID],
                                    start=(k == 0),
                                    stop=(k == KCH - 1),
                                )
                            nc.vector.tensor_scalar_mul(
                                g1loc[:, gb * HID:(gb + 1) * HID], ps[:],
                                dinv_sb[:, gb:gb + 1])
                            g1b = m1op.tile([P, HID], BF16,
                                            name=f"g1b{gb}", tag="g1b")
                            nc.vector.tensor_copy(
                                g1b[:], g1loc[:, gb * HID:(gb + 1) * HID])
                            nc.sync.dma_start(
                                g1_c[gb * P:(gb + 1) * P, 0:HID], g1b[:])

                # ---- AllGather layer-1 table (sliced; table layout
                # [slice, core, rows] so each collective output is contiguous)
                SL = SHP // NSL
                if DEBUG_LOCAL_AG:
                    nc.sync.dma_start(g1_full[0:SHP, :], g1_c[:])
                else:
                    for i in range(NSL):
                        nc.gpsimd.collective_compute(
                            "AllGather", mybir.AluOpType.bypass,
                            replica_groups=[list(range(NC))],
                            ins=[g1_c[i * SL:(i + 1) * SL, :]],
                            outs=[g1_full[i * NC * SL:(i + 1) * NC * SL, :]],
                        )

                # ---- layer 1 edge phase
                with (tc.tile_pool(name="e1idx", bufs=6) as idxp,
                      tc.tile_pool(name="e1msg", bufs=4) as msgp,
                      tc.tile_pool(name="e1S", bufs=4) as sp,
                      tc.tile_pool(name="e1ps", bufs=4, space="PSUM") as psp):
                    _emit_edge_phase(nc, tc, (idxp, msgp, sp, psp), meta,
                                     g1_full, acc1[:], HID, dst_sb, iota_sb,
                                     t_gidx, 1)

                # ---- layer-1 epilogue: out1 = relu(dinv*(acc+g1loc)+b1)
                a3 = acc1[:].rearrange("p (n h) -> p n h", h=HID)
                nc.vector.tensor_add(acc1[:], acc1[:], g1loc[:])
                nc.vector.tensor_tensor(
                    out=a3, in0=a3, in1=dinv_sb[:].to_broadcast([P, NBLK, HID]),
                    op=mybir.AluOpType.mult)
                nc.vector.tensor_tensor(
                    out=a3, in0=a3,
                    in1=b1_sb[:].to_broadcast([P, HID, NBLK]
                                              ).rearrange("p h n -> p n h"),
                    op=mybir.AluOpType.add)
                nc.vector.tensor_scalar_max(acc1[:], acc1[:], 0.0)

                # ---- transpose out1 -> o1t_d DRAM [HID, SHP]
                with (tc.tile_pool(name="tp", bufs=4, space="PSUM") as tpp,
                      tc.tile_pool(name="tpo", bufs=3) as tpo):
                    for gb in range(NBLK):
                        pst = tpp.tile([HID, P], F32, space="PSUM",
                                       name=f"pst{gb}", tag="pst")
                        nc.tensor.transpose(
                            pst[:], acc1[:, gb * HID:(gb + 1) * HID], ident_sb[:])
                        o1s = tpo.tile([HID, P], F32, name=f"o1s{gb}", tag="o1s")
                        nc.vector.tensor_copy(o1s[:], pst[:])
                        nc.sync.dma_start(o1t_d[:, gb * P:(gb + 1) * P], o1s[:])

            # ======== layer 2 scope ========
            with tc.tile_pool(name="l2", bufs=1) as l2p:
                g2loc = l2p.tile([P, NBLK * OUT], F32)
                acc2 = l2p.tile([P, NBLK * OUT], F32)
                nc.vector.memset(acc2[:], 0.0)

                # ---- layer-2 matmul: g2 = dinv * (out1 @ W2)
                with (tc.tile_pool(name="m2x", bufs=3) as o1xp,
                      tc.tile_pool(name="m2ps", bufs=4, space="PSUM") as m2psp,
                      tc.tile_pool(name="m2o", bufs=3) as m2op):
                    nwide = -(-SHP // WIDE)
                    for wi in range(nwide):
                        c0 = wi * WIDE
                        ncols = min(WIDE, SHP - c0)
                        o1w = o1xp.tile([HID, ncols], F32,
                                        name=f"o1w{wi}", tag="o1w")
                        nc.sync.dma_start(o1w[:], o1t_d[:, c0:c0 + ncols])
                        for rb in range(ncols // P):
                            gb = c0 // P + rb
                            ps2 = m2psp.tile([P, OUT], F32, space="PSUM",
                                             name=f"m2ps{gb}", tag="m2ps")
                            nc.tensor.matmul(
                                ps2[:], lhsT=o1w[:, rb * P:(rb + 1) * P],
                                rhs=w2_sb[:], start=True, stop=True)
                            nc.vector.tensor_scalar_mul(
                                g2loc[:, gb * OUT:(gb + 1) * OUT], ps2[:],
                                dinv_sb[:, gb:gb + 1])
                            g2b = m2op.tile([P, OUT], BF16,
                                            name=f"g2b{gb}", tag="g2b")
                            nc.vector.tensor_copy(
                                g2b[:], g2loc[:, gb * OUT:(gb + 1) * OUT])
                            nc.sync.dma_start(
                                g2_c[gb * P:(gb + 1) * P, 0:OUT], g2b[:])

                SL = SHP // NSL
                if DEBUG_LOCAL_AG:
                    nc.sync.dma_start(g2_full[0:SHP, :], g2_c[:])
                else:
                    for i in range(NSL):
                        nc.gpsimd.collective_compute(
                            "AllGather", mybir.AluOpType.bypass,
                            replica_groups=[list(range(NC))],
                            ins=[g2_c[i * SL:(i + 1) * SL, :]],
                            outs=[g2_full[i * NC * SL:(i + 1) * NC * SL, :]],
                        )

                # ---- layer 2 edge phase
                with (tc.tile_pool(name="e2idx", bufs=6) as idxp,
                      tc.tile_pool(name="e2msg", bufs=4) as msgp,
                      tc.tile_pool(name="e2S", bufs=4) as sp,
                      tc.tile_pool(name="e2ps", bufs=4, space="PSUM") as psp):
                    _emit_edge_phase(nc, tc, (idxp, msgp, sp, psp), meta,
                                     g2_full, acc2[:], OUT, dst_sb, iota_sb,
                                     t_gidx, 2)

                # ---- layer-2 epilogue: out = dinv*(acc2+g2loc)+b2
                c3 = acc2[:].rearrange("p (n h) -> p n h", h=OUT)
                nc.vector.tensor_add(acc2[:], acc2[:], g2loc[:])
                nc.vector.tensor_tensor(
                    out=c3, in0=c3, in1=dinv_sb[:].to_broadcast([P, NBLK, OUT]),
                    op=mybir.AluOpType.mult)
                nc.vector.tensor_tensor(
                    out=c3, in0=c3,
                    in1=b2_sb[:].to_broadcast([P, OUT, NBLK]
                                              ).rearrange("p h n -> p n h"),
                    op=mybir.AluOpType.add)
                for gb in range(NBLK):
                    nc.sync.dma_start(
                        t_out[gb * P:(gb + 1) * P, :],
                        acc2[:, gb * OUT:(gb + 1) * OUT])

    nc.compile()
    return nc


# ------------------------------------------------------------------ frontend

_CACHE = {}


def run(trace=False, **inputs):
    in_maps, meta = _host_prep(
        inputs["x"], inputs["edge_index"], inputs["W1"], inputs["b1"],
        inputs["W2"], inputs["b2"])
    key = (meta["N"], meta["IN_DIM"], meta["HID"], meta["OUT"], meta["TTOT"],
           tuple((s, p, n, tuple(sg)) for s, p, n, sg in meta["chunks"]))
    if key not in _CACHE:
        _CACHE.clear()
        _CACHE[key] = _build_program(meta)
    nc = _CACHE[key]
    res = run_bass_kernel_spmd(nc, in_maps, list(range(NC)), trace=trace)
    SH = meta["SH"]
    out = np.concatenate([res.results[c]["out"][:SH] for c in range(NC)], axis=0)
    return out.astype(np.float32), res


def kernel(**inputs):
    out, _ = run(trace=False, **inputs)
    return out
